# revision 3
# baseline (speedup 1.0000x reference)
"""nn_SamplingLoss kernel: data-parallel over points across 8 NeuronCores.

Shards xyz/rgb/pcd_weight along N, replicates the panorama + pose params,
computes the masked weighted loss per shard on each core, and reduces the
masked sum / mask count across cores (final division on host).
"""
import numpy as np

N_PTS = 2_000_000
IMG_H, IMG_W = 1024, 2048
N_CORES = 8
PI = np.pi


def _impl_jax(translation, yaw, pitch, roll, xyz, rgb, img, img_weight, pcd_weight):
    import jax
    import jax.numpy as jnp
    from jax.sharding import Mesh, PartitionSpec as P
    from jax.experimental.shard_map import shard_map

    devices = jax.devices()[:N_CORES]
    mesh = Mesh(np.asarray(devices), ("core",))

    def per_shard(xyz_s, rgb_s, pw_s, rgbw_f, R_f, t_f):
        new_xyz = (R_f @ (xyz_s.T - t_f)).T
        phi = jnp.arctan2(new_xyz[:, 1], new_xyz[:, 0]) + PI
        theta = jnp.arctan2(
            jnp.sqrt(new_xyz[:, 0] ** 2 + new_xyz[:, 1] ** 2), new_xyz[:, 2]
        )
        cx = 2.0 * (1.0 - phi / (2.0 * PI)) - 1.0
        cy = 2.0 * (theta / PI) - 1.0
        x = jnp.clip((cx + 1.0) * 0.5 * IMG_W - 0.5, 0.0, IMG_W - 1.0)
        y = jnp.clip((cy + 1.0) * 0.5 * IMG_H - 0.5, 0.0, IMG_H - 1.0)
        x0 = jnp.floor(x)
        y0 = jnp.floor(y)
        wx = (x - x0)[:, None]
        wy = (y - y0)[:, None]
        x0i = x0.astype(jnp.int32)
        y0i = y0.astype(jnp.int32)
        x1i = jnp.minimum(x0i + 1, IMG_W - 1)
        y1i = jnp.minimum(y0i + 1, IMG_H - 1)

        def samp(im):
            v00 = im[y0i, x0i]
            v01 = im[y0i, x1i]
            v10 = im[y1i, x0i]
            v11 = im[y1i, x1i]
            return (v00 * (1 - wx) * (1 - wy) + v01 * wx * (1 - wy)
                    + v10 * (1 - wx) * wy + v11 * wx * wy)

        sample = samp(rgbw_f)
        sample_rgb = sample[:, :3]
        raw_loss = jnp.sqrt(jnp.sum((sample_rgb - rgb_s) ** 2, axis=-1))
        w_img = sample[:, 3]
        raw_loss = 0.5 * (w_img + pw_s) * raw_loss
        mask = (jnp.sum(sample_rgb == 0, axis=1) != 3).astype(jnp.float32)
        s = jnp.sum(raw_loss * mask)
        c = jnp.sum(mask)
        out = jnp.stack([s, c])
        return jax.lax.psum(out, "core")

    # rotation matrix built exactly like the reference (f32 ops, on host)
    cy_, sy_ = np.cos(yaw.astype(np.float32))[0], np.sin(yaw.astype(np.float32))[0]
    cp, sp = np.cos(pitch.astype(np.float32))[0], np.sin(pitch.astype(np.float32))[0]
    cr, sr = np.cos(roll.astype(np.float32))[0], np.sin(roll.astype(np.float32))[0]
    RX = np.array([[1, 0, 0], [0, cr, -sr], [0, sr, cr]], dtype=np.float32)
    RY = np.array([[cp, 0, sp], [0, 1, 0], [-sp, 0, cp]], dtype=np.float32)
    RZ = np.array([[cy_, -sy_, 0], [sy_, cy_, 0], [0, 0, 1]], dtype=np.float32)
    R = (RZ @ RY @ RX).astype(np.float32)

    fn = jax.jit(
        shard_map(
            per_shard,
            mesh=mesh,
            in_specs=(P("core"), P("core"), P("core"), P(), P(), P()),
            out_specs=P(),
            check_rep=False,
        )
    )
    rgbw = np.concatenate([img, img_weight], axis=2)
    out = np.asarray(fn(xyz, rgb, pcd_weight, rgbw, R, translation))
    return np.float32(out[0] / out[1])


def kernel(translation, yaw, pitch, roll, xyz, rgb, img, img_weight, pcd_weight):
    return _impl_jax(
        np.asarray(translation), np.asarray(yaw), np.asarray(pitch),
        np.asarray(roll), np.asarray(xyz), np.asarray(rgb), np.asarray(img),
        np.asarray(img_weight), np.asarray(pcd_weight),
    )
